# revision 19
# baseline (speedup 1.0000x reference)
"""Int8-quantized matmul (dynamic per-tensor abs-max calibration) on 8 TRN2 cores.

Reference semantics (all fp32 unless noted):
    ls = 127 / max(|lhs|max, 1e-12);  rs = 127 / max(|rhs|max, 1e-12)
    ql = round(lhs*ls) clipped to [-127,127]  (int8)
    qr = round(rhs*rs) clipped to [-127,127]  (int8)
    out = (ql @ qr, int32 accumulation) / (ls*rs)

Device strategy (2 row-groups x 4 col-groups = 8 cores):
  - core i: rows block ri = i//4 of lhs (as lhsT, pre-transposed on host),
    cols block ci = i%4 of rhs.  Each core computes out block [2048, 1024].
  - The device program is identical on every core; per-core differences are
    folded into host-side input permutations:
      * k axis rolled so the core's "own" 1/8-of-rhs k-half is k-tiles 0..15
        (both lhsT and rhs rolled identically; contraction is k-order
        invariant),
      * lhsT columns permuted so the core's 1/8-of-lhs stats slice is
        columns 0..511 (output rows un-permuted on the host at gather).
  - calibration: the lhs stats slice (lhsT cols 0:512) is DMA'd once into
    SBUF, abs-max-reduced, KEPT resident, and later quantized in place
    into the first two m-macros.  The rhs stats slice (k-tiles 0..15) is
    reduced from streaming chunks and re-read once after calibration.
    Each side's global amax is an AllGather of the partition-reduced
    per-core max; the lhs collective is issued before the rhs stats
    stream so its latency hides under DMA, and the resident weight
    macros quantize (needing only ls) inside the rhs collective window.
  - quantized values are kept on the int8 grid but stored as bf16 (exact
    for |q| <= 127); PE matmul accumulates in fp32.
  - round-half-to-even via the magic constant: q = ((x*s)+1.5*2^23)-1.5*2^23.
  - k-tiles are consumed in order 0..31 = local-rhs-half first, so the PE
    never waits on the remote rhs half still streaming in.  ACT does the
    scale-multiply pass, DVE the round+cast pass and dequant; outputs
    leave on the Pool queue.

kernel(lhs, rhs) takes the FULL fp32 inputs and returns the FULL [4096,4096]
fp32 output.
"""

import numpy as np

P = 128
K = 4096
M = 4096
N = 4096
RG = 2            # row groups (lhs)
CG = 4            # col groups (rhs)
MB = M // RG      # 2048 rows of out per core
NB = N // CG      # 1024 cols of out per core
KT = K // P       # 32 k-tiles
KH = KT // 2      # 16: k-tiles in the local (stats) half
MACRO = 256       # lhsT macro-tile (m columns per quantize/matmul step)
NMACRO = MB // MACRO  # 8
MAGIC = 12582912.0    # 1.5 * 2^23: (t + MAGIC) - MAGIC == round-half-even(t)
N_CORES = 8

_cached = None


def _build_program():
    """Build the SPMD Bass program once; returns the compiled Bacc."""
    from contextlib import ExitStack

    import concourse.bass as bass
    import concourse.mybir as mybir
    import concourse.tile as tile
    from concourse import bacc, bass_isa

    f32 = mybir.dt.float32
    bf16 = mybir.dt.bfloat16

    nc = bacc.Bacc(
        "TRN2",
        target_bir_lowering=False,
        debug=False,
        num_devices=N_CORES,
    )

    lhsT = nc.dram_tensor("lhsT", [K, MB], f32, kind="ExternalInput").ap()
    rhs = nc.dram_tensor("rhs", [K, NB], f32, kind="ExternalInput").ap()
    out = nc.dram_tensor("out", [MB, NB], f32, kind="ExternalOutput").ap()

    rhs_v = rhs.rearrange("(t p) n -> p t n", p=P)     # [128, 32, 1024]
    lhsT_v = lhsT.rearrange("(t p) m -> p t m", p=P)   # [128, 32, 2048]
    out_v = out.rearrange("(mt p) n -> mt p n", p=P)   # [16, 128, 1024]

    AX = mybir.AxisListType
    OP = mybir.AluOpType

    with tile.TileContext(nc) as tc, ExitStack() as ctx:
        singles = ctx.enter_context(tc.tile_pool(name="singles", bufs=1))
        lexcp = ctx.enter_context(tc.tile_pool(name="lexcp", bufs=1))
        psum = ctx.enter_context(tc.tile_pool(name="psum", bufs=8, space="PSUM"))
        dram = ctx.enter_context(tc.tile_pool(name="ccdram", bufs=1, space="DRAM"))

        stats = singles.tile([P, 2, 9], f32)           # per-chunk |max|es
        qr_all = singles.tile([P, KT, NB], bf16)       # 64KB/part
        lexc = lexcp.tile([P, KT, 512], f32)           # 64KB/part, kept

        def scale_from(amax_col, sc_out):
            """sc_out = 127/amax via DVE reciprocal + one Newton step.
            (reference clamps amax at 1e-12; |randn| max over 16M samples is
            ~5, so the clamp is a provable no-op for this input spec)"""
            r_t = singles.tile([P, 1], f32)
            t_t = singles.tile([P, 1], f32)
            nc.vector.reciprocal(r_t, amax_col)
            nc.vector.tensor_mul(t_t, amax_col, r_t)
            nc.vector.tensor_scalar(t_t, t_t, -1.0, 2.0, op0=OP.mult, op1=OP.add)
            nc.vector.tensor_mul(r_t, r_t, t_t)
            nc.vector.tensor_scalar_mul(sc_out, r_t, 127.0)

        def cc_issue(side):
            """Partition-reduce stats[:, side, :] to one scalar, AllGather
            the 8 per-core scalars; returns the [P, 8] broadcast readback."""
            pp = singles.tile([P, 1], f32, name=f"pp{side}")
            nslot = 8 if side == 0 else 9
            nc.vector.tensor_reduce(
                out=pp, in_=stats[:, side, 0:nslot], axis=AX.X, op=OP.max
            )
            al = singles.tile([P, 1], f32, name=f"al{side}")
            nc.gpsimd.partition_all_reduce(
                al, pp, channels=P, reduce_op=bass_isa.ReduceOp.max
            )
            cc_in = dram.tile([1, 1], f32, name=f"cci{side}")
            cc_out = dram.tile([N_CORES, 1], f32, name=f"cco{side}")
            nc.gpsimd.dma_start(out=cc_in[0:1, 0:1], in_=al[0:1, 0:1])
            nc.gpsimd.collective_compute(
                "AllGather",
                OP.bypass,
                replica_groups=[list(range(N_CORES))],
                ins=[cc_in[:, :]],
                outs=[cc_out[:, :]],
            )
            g128 = singles.tile([P, N_CORES], f32, name=f"g{side}")
            bcast_ap = bass.AP(
                tensor=cc_out.tensor,
                offset=cc_out.offset,
                ap=[[0, P], [1, N_CORES]],
            )
            nc.gpsimd.dma_start(out=g128, in_=bcast_ap)
            return g128

        def cc_finish(g128, sc_out, side):
            gmax = singles.tile([P, 1], f32, name=f"gm{side}")
            nc.vector.tensor_reduce(out=gmax, in_=g128, axis=AX.X, op=OP.max)
            scale_from(gmax, sc_out)

        lsrs = singles.tile([P, 2], f32)
        ls_bc = lsrs[:, 0:1]
        rs_bc = lsrs[:, 1:2]

        # ---------------- lhs stats into resident lexc + its collective ---
        p2r = ctx.enter_context(tc.tile_pool(name="p2r", bufs=3))
        qtmp = ctx.enter_context(tc.tile_pool(name="qtmp", bufs=2))
        qlp = ctx.enter_context(tc.tile_pool(name="qlp", bufs=2))
        outp = ctx.enter_context(tc.tile_pool(name="outp", bufs=2))
        for j in range(8):
            nc.sync.dma_start(
                out=lexc[:, 4 * j : 4 * (j + 1), :],
                in_=lhsT_v[:, 4 * j : 4 * (j + 1), 0:512],
            )
            nc.vector.tensor_reduce(
                out=stats[:, 0, j : j + 1],
                in_=lexc[:, 4 * j : 4 * (j + 1), :],
                axis=AX.XY,
                op=OP.max,
                apply_absolute_value=True,
            )
        gl = cc_issue(0)

        # ---------------- rhs stats (k-tiles 0..15) + its collective ------
        for j in range(7):
            ch = p2r.tile([P, 2, NB], f32, tag="st")
            nc.sync.dma_start(out=ch, in_=rhs_v[:, 2 * j : 2 * (j + 1), :])
            nc.vector.tensor_reduce(
                out=stats[:, 1, j : j + 1],
                in_=ch,
                axis=AX.XY,
                op=OP.max,
                apply_absolute_value=True,
            )
        # last two k-tiles as 1-k-tile pieces: halves the trailing reduce
        # on the rhs collective's critical chain
        ch = p2r.tile([P, 2, NB], f32, tag="st")
        nc.sync.dma_start(out=ch, in_=rhs_v[:, 14:16, :])
        nc.vector.tensor_reduce(
            out=stats[:, 1, 7:8], in_=ch[:, 0:1, :], axis=AX.XY, op=OP.max,
            apply_absolute_value=True,
        )
        nc.vector.tensor_reduce(
            out=stats[:, 1, 8:9], in_=ch[:, 1:2, :], axis=AX.XY, op=OP.max,
            apply_absolute_value=True,
        )
        # hoist the first 3 local-half qr re-read DMAs ahead of the rhs
        # collective: they prefetch into p2r during its latency window
        # (their quantize is emitted after rs below).
        qr_pre = []
        for c in range(3):
            rf = p2r.tile([P, 2, NB], f32, tag="st")
            nc.sync.dma_start(out=rf, in_=rhs_v[:, 2 * c : 2 * (c + 1), :])
            qr_pre.append(rf)
        gr = cc_issue(1)

        # ls is ready while the rhs collective is still in flight; the
        # resident-lexc macros quantize inside that window.
        cc_finish(gl, ls_bc, 0)

        # ---------------- quantize + matmul --------------------------------
        def quant(dst, src, scale_ap):
            """pass1 (ACT): t = src*scale; pass2 (DVE): round + cast bf16."""
            tq = qtmp.tile([P, 2048], f32, tag="tq")
            s_ap = tq[:, 0 : src.free_size()].rearrange(
                "p (a b) -> p a b", a=src.shape[1]
            )
            nc.scalar.mul(out=s_ap, in_=src, mul=scale_ap)
            nc.vector.tensor_scalar(
                out=dst, in0=s_ap, scalar1=MAGIC, scalar2=-MAGIC,
                op0=OP.add, op1=OP.add,
            )

        def ql_tile(mt):
            return qlp.tile([P, KT, MACRO], bf16, tag="ql", name=f"ql{mt}")

        def ql_chunk_resident(qlt, mt, c):
            # quantize lexc[:, 8c:8c+8, mt*256:(mt+1)*256] -> qlt
            quant(
                qlt[:, 8 * c : 8 * (c + 1), :],
                lexc[:, 8 * c : 8 * (c + 1), mt * MACRO : (mt + 1) * MACRO],
                ls_bc,
            )

        def ql_chunk_stream(qlt, mt, j):
            lf = p2r.tile([P, 8, MACRO], f32, tag="st")
            nc.sync.dma_start(
                out=lf,
                in_=lhsT_v[:, 8 * j : 8 * (j + 1), mt * MACRO : (mt + 1) * MACRO],
            )
            quant(qlt[:, 8 * j : 8 * (j + 1), :], lf, ls_bc)

        def qr_chunk(c):
            rf = p2r.tile([P, 2, NB], f32, tag="st")
            nc.sync.dma_start(out=rf, in_=rhs_v[:, 2 * c : 2 * (c + 1), :])
            quant(qr_all[:, 2 * c : 2 * (c + 1), :], rf, rs_bc)

        # m0/m1 weights quantize from resident lexc inside the rhs
        # collective window (only ls is needed; no DMA involved).
        ql0 = ql_tile(0)
        ql1 = ql_tile(1)
        for c in range(2):
            ql_chunk_resident(ql0, 0, c)
        for c in range(2):
            ql_chunk_resident(ql1, 1, c)
        for c in range(2, 4):
            ql_chunk_resident(ql0, 0, c)
        for c in range(2, 4):
            ql_chunk_resident(ql1, 1, c)

        # PE warm-up bridge: matmuls on already-quantized (garbage-free)
        # weight data into a scratch psum bank, spanning the window between
        # the weight quantize and the first real matmul so the PE clock is
        # fully ramped when the burst starts.  Results are never read.
        wps = psum.tile([P, 512], f32, tag="ps", name="warm")
        for w in range(60):
            nc.tensor.matmul(
                wps[:, 0:256],
                lhsT=ql1[:, 31, 0:P],
                rhs=ql1[:, w % KT, :],
                start=True,
                stop=True,
            )

        # rs, then the local-half qr re-read (first 3 chunks prefetched)
        cc_finish(gr, rs_bc, 1)
        for h in range(2):
            quant(
                qr_all[:, h : h + 1, :], qr_pre[0][:, h : h + 1, :], rs_bc
            )
        for c in range(1, 3):
            quant(qr_all[:, 2 * c : 2 * (c + 1), :], qr_pre[c], rs_bc)
        for c in range(3, 8):
            qr_chunk(c)

        # d = 1/(ls*rs), Newton-polished (first consumed by macro-0 dequant)
        p_t = singles.tile([P, 1], f32)
        d_t = singles.tile([P, 1], f32)
        u_t = singles.tile([P, 1], f32)
        nc.vector.tensor_mul(p_t, lsrs[:, 0:1], lsrs[:, 1:2])
        nc.vector.reciprocal(d_t, p_t)
        nc.vector.tensor_mul(u_t, p_t, d_t)
        nc.vector.tensor_scalar(u_t, u_t, -1.0, 2.0, op0=OP.mult, op1=OP.add)
        nc.vector.tensor_mul(d_t, d_t, u_t)
        d_bc = d_t[:, 0:1]

        # --- matmul helpers ---
        def mk_psum(m):
            return [
                psum.tile([P, 512], f32, tag="ps", name=f"ps{m}_{q}")
                for q in range(4)
            ]

        def mm_k(ql, pst, k, st, sp):
            for ms in range(2):
                w = ql[:, k, ms * P : (ms + 1) * P]
                nc.tensor.matmul(
                    pst[2 * ms], lhsT=w, rhs=qr_all[:, k, 0:512],
                    start=st, stop=sp,
                )
                nc.tensor.matmul(
                    pst[2 * ms + 1], lhsT=w, rhs=qr_all[:, k, 512:1024],
                    start=st, stop=sp,
                )

        def dequant_out(pst, m, ms_range=(0, 1), eng=None):
            eng = eng or nc.gpsimd
            for ms in ms_range:
                for h in range(2):
                    osb = outp.tile([P, 512], f32)
                    nc.vector.tensor_scalar_mul(osb, pst[2 * ms + h], d_bc)
                    eng.dma_start(
                        out=out_v[m * 2 + ms, :, 512 * h : 512 * (h + 1)],
                        in_=osb,
                    )

        # --- m0/m1: local k-half first, then remote with the qr stream ---
        pst0 = mk_psum(0)
        for k in range(KH):
            mm_k(ql0, pst0, k, k == 0, False)
        pst1 = mk_psum(1)
        for k in range(KH):
            mm_k(ql1, pst1, k, k == 0, False)

        # remote rhs half
        for c in range(8, 16):
            qr_chunk(c)

        for k in range(KH, KT):
            mm_k(ql0, pst0, k, False, k == KT - 1)
        for k in range(KH, KT):
            mm_k(ql1, pst1, k, False, k == KT - 1)

        # --- macros 2..6: stream lhsT, quantize, matmul.  Each macro's
        # weight quantize is emitted BEFORE the previous macros' dequant so
        # the DVE serves the PE's critical input first; psum stays within 8
        # banks because the deferred dequant still precedes the next
        # macro's matmuls.
        pending = [(pst0, 0), (pst1, 1)]
        for mt in range(2, NMACRO - 1):
            ql = ql_tile(mt)
            for j in range(4):
                ql_chunk_stream(ql, mt, j)
            for pq, pm in pending:
                dequant_out(pq, pm, eng=nc.sync if pm >= NMACRO - 2 else None)
            pending = []
            pst = mk_psum(mt)
            for k in range(KT):
                mm_k(ql, pst, k, k == 0, k == KT - 1)
            pending.append((pst, mt))

        # --- macro 7: the two output halves run sequentially so the first
        # half's dequant + store hides under the second half's matmuls ---
        mt = NMACRO - 1
        ql = ql_tile(mt)
        for j in range(4):
            ql_chunk_stream(ql, mt, j)
        for pq, pm in pending:
            dequant_out(pq, pm, eng=nc.sync)
        pending = []
        pst = mk_psum(mt)
        for k in range(KT):
            ms = 0
            w = ql[:, k, ms * P : (ms + 1) * P]
            nc.tensor.matmul(pst[0], lhsT=w, rhs=qr_all[:, k, 0:512],
                             start=k == 0, stop=k == KT - 1)
            nc.tensor.matmul(pst[1], lhsT=w, rhs=qr_all[:, k, 512:1024],
                             start=k == 0, stop=k == KT - 1)
        dequant_out(pst, mt, ms_range=(0,), eng=nc.sync)
        for k in range(KT):
            ms = 1
            w = ql[:, k, ms * P : (ms + 1) * P]
            nc.tensor.matmul(pst[2], lhsT=w, rhs=qr_all[:, k, 0:512],
                             start=k == 0, stop=k == KT - 1)
            nc.tensor.matmul(pst[3], lhsT=w, rhs=qr_all[:, k, 512:1024],
                             start=k == 0, stop=k == KT - 1)
        dequant_out(pst, mt, ms_range=(1,), eng=nc.sync)

    nc.compile()
    return nc


def _get_program():
    global _cached
    if _cached is None:
        _cached = _build_program()
    return _cached


def _mperm(ci):
    sl = ci * 512
    return np.concatenate(
        [
            np.arange(sl, sl + 512),
            np.arange(0, sl),
            np.arange(sl + 512, MB),
        ]
    )


def _shard_inputs(lhs, rhs):
    lhs = np.ascontiguousarray(np.asarray(lhs, dtype=np.float32))
    rhs = np.ascontiguousarray(np.asarray(rhs, dtype=np.float32))
    assert lhs.shape == (M, K) and rhs.shape == (K, N)
    lhsT = np.ascontiguousarray(lhs.T)  # [K, M]
    in_maps = []
    for i in range(N_CORES):
        ri, ci = divmod(i, CG)
        lT = lhsT[:, ri * MB : (ri + 1) * MB]
        rsh = rhs[:, ci * NB : (ci + 1) * NB]
        # roll k so the core's stats k-half (rows [ri*MB,(ri+1)*MB)) is first
        if ri:
            lT = np.concatenate([lT[MB:], lT[:MB]], axis=0)
            rsh = np.concatenate([rsh[MB:], rsh[:MB]], axis=0)
        # permute lhsT cols so the core's stats slice (ci-th 512) is first
        lT = np.ascontiguousarray(lT[:, _mperm(ci)])
        rsh = np.ascontiguousarray(rsh)
        in_maps.append({"lhsT": lT, "rhs": rsh})
    return in_maps


def _gather(results):
    out = np.empty((M, N), dtype=np.float32)
    for i in range(N_CORES):
        ri, ci = divmod(i, CG)
        rows = ri * MB + _mperm(ci)
        out[rows, ci * NB : (ci + 1) * NB] = results[i]["out"]
    return out


def run(lhs, rhs, trace=False):
    """Run the kernel; returns (out, BassKernelResults)."""
    from concourse import bass_utils

    nc = _get_program()
    in_maps = _shard_inputs(lhs, rhs)
    res = bass_utils.run_bass_kernel_spmd(
        nc, in_maps, core_ids=list(range(N_CORES)), trace=trace
    )
    return _gather(res.results), res


def kernel(lhs, rhs):
    out, _ = run(lhs, rhs, trace=False)
    return out


# revision 33
# speedup vs baseline: 1.0084x; 1.0084x over previous
"""Int8-quantized matmul (dynamic per-tensor abs-max calibration) on 8 TRN2 cores.

Reference semantics (all fp32 unless noted):
    ls = 127 / max(|lhs|max, 1e-12);  rs = 127 / max(|rhs|max, 1e-12)
    ql = round(lhs*ls) clipped to [-127,127]  (int8)
    qr = round(rhs*rs) clipped to [-127,127]  (int8)
    out = (ql @ qr, int32 accumulation) / (ls*rs)

Device strategy (2 row-groups x 4 col-groups = 8 cores):
  - core i: rows block ri = i//4 of lhs (as lhsT, pre-transposed on host),
    cols block ci = i%4 of rhs.  Each core computes out block [2048, 1024].
  - The device program is identical on every core; per-core differences are
    folded into host-side input permutations:
      * k axis rolled so the core's "own" 1/8-of-rhs k-half is k-tiles 0..15
        (both lhsT and rhs rolled identically; contraction is k-order
        invariant),
      * lhsT columns permuted so the core's 1/8-of-lhs stats slice is
        columns 0..511 (output rows un-permuted on the host at gather).
  - calibration: the lhs stats slice (lhsT cols 0:512) is DMA'd once into
    SBUF, abs-max-reduced, KEPT resident, and later quantized in place
    into the first two m-macros.  The rhs stats slice (k-tiles 0..15) is
    reduced from streaming chunks and re-read once after calibration.
    Each side's global amax is an AllGather of the partition-reduced
    per-core max; the lhs collective is issued before the rhs stats
    stream so its latency hides under DMA, and the resident weight
    macros quantize (needing only ls) inside the rhs collective window.
  - quantized values are kept on the int8 grid but stored as bf16 (exact
    for |q| <= 127); PE matmul accumulates in fp32.
  - round-half-to-even via the magic constant: q = ((x*s)+1.5*2^23)-1.5*2^23.
  - k-tiles are consumed in order 0..31 = local-rhs-half first, so the PE
    never waits on the remote rhs half still streaming in.  ACT does the
    scale-multiply pass, DVE the round+cast pass and dequant; outputs
    leave on the Pool queue.

kernel(lhs, rhs) takes the FULL fp32 inputs and returns the FULL [4096,4096]
fp32 output.
"""

import numpy as np

P = 128
K = 4096
M = 4096
N = 4096
RG = 2            # row groups (lhs)
CG = 4            # col groups (rhs)
MB = M // RG      # 2048 rows of out per core
NB = N // CG      # 1024 cols of out per core
KT = K // P       # 32 k-tiles
KH = KT // 2      # 16: k-tiles in the local (stats) half
MACRO = 256       # lhsT macro-tile (m columns per quantize/matmul step)
NMACRO = MB // MACRO  # 8
MAGIC = 12582912.0    # 1.5 * 2^23: (t + MAGIC) - MAGIC == round-half-even(t)
N_CORES = 8

_cached = None


def _build_program():
    """Build the SPMD Bass program once; returns the compiled Bacc."""
    from contextlib import ExitStack

    import concourse.bass as bass
    import concourse.mybir as mybir
    import concourse.tile as tile
    from concourse import bacc, bass_isa

    f32 = mybir.dt.float32
    bf16 = mybir.dt.bfloat16

    nc = bacc.Bacc(
        "TRN2",
        target_bir_lowering=False,
        debug=False,
        num_devices=N_CORES,
    )

    lhsT = nc.dram_tensor("lhsT", [K, MB], f32, kind="ExternalInput").ap()
    rhs = nc.dram_tensor("rhs", [K, NB], f32, kind="ExternalInput").ap()
    out = nc.dram_tensor("out", [MB, NB], f32, kind="ExternalOutput").ap()

    rhs_v = rhs.rearrange("(t p) n -> p t n", p=P)     # [128, 32, 1024]
    lhsT_v = lhsT.rearrange("(t p) m -> p t m", p=P)   # [128, 32, 2048]
    out_v = out.rearrange("(mt p) n -> mt p n", p=P)   # [16, 128, 1024]

    AX = mybir.AxisListType
    OP = mybir.AluOpType

    with tile.TileContext(nc) as tc, ExitStack() as ctx:
        singles = ctx.enter_context(tc.tile_pool(name="singles", bufs=1))
        lexcp = ctx.enter_context(tc.tile_pool(name="lexcp", bufs=1))
        psum = ctx.enter_context(tc.tile_pool(name="psum", bufs=8, space="PSUM"))
        dram = ctx.enter_context(tc.tile_pool(name="ccdram", bufs=1, space="DRAM"))

        stats = singles.tile([P, 2, 9], f32)           # per-chunk |max|es
        qr_all = singles.tile([P, KT, NB], bf16)       # 64KB/part
        lexc = lexcp.tile([P, KT, 512], f32)           # 64KB/part, kept

        def scale_from(amax_col, sc_out):
            """sc_out = 127/amax via DVE reciprocal + one Newton step.
            (reference clamps amax at 1e-12; |randn| max over 16M samples is
            ~5, so the clamp is a provable no-op for this input spec)"""
            r_t = singles.tile([P, 1], f32)
            t_t = singles.tile([P, 1], f32)
            nc.vector.reciprocal(r_t, amax_col)
            nc.vector.tensor_mul(t_t, amax_col, r_t)
            nc.vector.tensor_scalar(t_t, t_t, -1.0, 2.0, op0=OP.mult, op1=OP.add)
            nc.vector.tensor_mul(r_t, r_t, t_t)
            nc.vector.tensor_scalar_mul(sc_out, r_t, 127.0)

        def cc_issue(side):
            """Partition-reduce stats[:, side, :] to one scalar, AllGather
            the 8 per-core scalars; returns the [P, 8] broadcast readback."""
            pp = singles.tile([P, 1], f32, name=f"pp{side}")
            nslot = 8 if side == 0 else 9
            nc.vector.tensor_reduce(
                out=pp, in_=stats[:, side, 0:nslot], axis=AX.X, op=OP.max
            )
            al = singles.tile([P, 1], f32, name=f"al{side}")
            nc.gpsimd.partition_all_reduce(
                al, pp, channels=P, reduce_op=bass_isa.ReduceOp.max
            )
            cc_in = dram.tile([1, 1], f32, name=f"cci{side}")
            cc_out = dram.tile([N_CORES, 1], f32, name=f"cco{side}")
            nc.gpsimd.dma_start(out=cc_in[0:1, 0:1], in_=al[0:1, 0:1])
            nc.gpsimd.collective_compute(
                "AllGather",
                OP.bypass,
                replica_groups=[list(range(N_CORES))],
                ins=[cc_in[:, :]],
                outs=[cc_out[:, :]],
            )
            g128 = singles.tile([P, N_CORES], f32, name=f"g{side}")
            bcast_ap = bass.AP(
                tensor=cc_out.tensor,
                offset=cc_out.offset,
                ap=[[0, P], [1, N_CORES]],
            )
            nc.gpsimd.dma_start(out=g128, in_=bcast_ap)
            return g128

        def cc_finish(g128, sc_out, side):
            gmax = singles.tile([P, 1], f32, name=f"gm{side}")
            nc.vector.tensor_reduce(out=gmax, in_=g128, axis=AX.X, op=OP.max)
            scale_from(gmax, sc_out)

        lsrs = singles.tile([P, 2], f32)
        ls_bc = lsrs[:, 0:1]
        rs_bc = lsrs[:, 1:2]

        # ---------------- lhs stats into resident lexc + its collective ---
        p2r = ctx.enter_context(tc.tile_pool(name="p2r", bufs=3))
        qtmp = ctx.enter_context(tc.tile_pool(name="qtmp", bufs=2))
        qlp = ctx.enter_context(tc.tile_pool(name="qlp", bufs=2))
        outp = ctx.enter_context(tc.tile_pool(name="outp", bufs=2))
        for j in range(8):
            nc.sync.dma_start(
                out=lexc[:, 4 * j : 4 * (j + 1), :],
                in_=lhsT_v[:, 4 * j : 4 * (j + 1), 0:512],
            )
            nc.vector.tensor_reduce(
                out=stats[:, 0, j : j + 1],
                in_=lexc[:, 4 * j : 4 * (j + 1), :],
                axis=AX.XY,
                op=OP.max,
                apply_absolute_value=True,
            )
        gl = cc_issue(0)

        # ---------------- rhs stats (k-tiles 0..15) + its collective ------
        for j in range(7):
            ch = p2r.tile([P, 2, NB], f32, tag="st")
            nc.sync.dma_start(out=ch, in_=rhs_v[:, 2 * j : 2 * (j + 1), :])
            nc.vector.tensor_reduce(
                out=stats[:, 1, j : j + 1],
                in_=ch,
                axis=AX.XY,
                op=OP.max,
                apply_absolute_value=True,
            )
        # last two k-tiles as 1-k-tile pieces: halves the trailing reduce
        # on the rhs collective's critical chain
        ch = p2r.tile([P, 2, NB], f32, tag="st")
        nc.sync.dma_start(out=ch, in_=rhs_v[:, 14:16, :])
        nc.vector.tensor_reduce(
            out=stats[:, 1, 7:8], in_=ch[:, 0:1, :], axis=AX.XY, op=OP.max,
            apply_absolute_value=True,
        )
        nc.vector.tensor_reduce(
            out=stats[:, 1, 8:9], in_=ch[:, 1:2, :], axis=AX.XY, op=OP.max,
            apply_absolute_value=True,
        )
        # hoist the first 3 local-half qr re-read DMAs ahead of the rhs
        # collective: they prefetch into p2r during its latency window
        # (their quantize is emitted after rs below).
        qr_pre = []
        for c in range(3):
            rf = p2r.tile([P, 2, NB], f32, tag="st")
            nc.sync.dma_start(out=rf, in_=rhs_v[:, 2 * c : 2 * (c + 1), :])
            qr_pre.append(rf)
        gr = cc_issue(1)

        # ls is ready while the rhs collective is still in flight; the
        # resident-lexc macros quantize inside that window.
        cc_finish(gl, ls_bc, 0)

        # ---------------- quantize + matmul --------------------------------
        def quant(dst, src, scale_ap):
            """pass1 (ACT): t = src*scale; pass2 (DVE): round + cast bf16."""
            tq = qtmp.tile([P, 2048], f32, tag="tq")
            s_ap = tq[:, 0 : src.free_size()].rearrange(
                "p (a b) -> p a b", a=src.shape[1]
            )
            nc.scalar.mul(out=s_ap, in_=src, mul=scale_ap)
            nc.vector.tensor_scalar(
                out=dst, in0=s_ap, scalar1=MAGIC, scalar2=-MAGIC,
                op0=OP.add, op1=OP.add,
            )

        def ql_tile(mt):
            return qlp.tile([P, KT, MACRO], bf16, tag="ql", name=f"ql{mt}")

        def ql_chunk_resident(qlt, mt, c):
            # quantize lexc[:, 8c:8c+8, mt*256:(mt+1)*256] -> qlt
            quant(
                qlt[:, 8 * c : 8 * (c + 1), :],
                lexc[:, 8 * c : 8 * (c + 1), mt * MACRO : (mt + 1) * MACRO],
                ls_bc,
            )

        def ql_chunk_stream(qlt, mt, j):
            lf = p2r.tile([P, 8, MACRO], f32, tag="st")
            nc.sync.dma_start(
                out=lf,
                in_=lhsT_v[:, 8 * j : 8 * (j + 1), mt * MACRO : (mt + 1) * MACRO],
            )
            quant(qlt[:, 8 * j : 8 * (j + 1), :], lf, ls_bc)

        def qr_chunk(c):
            rf = p2r.tile([P, 2, NB], f32, tag="st")
            nc.sync.dma_start(out=rf, in_=rhs_v[:, 2 * c : 2 * (c + 1), :])
            quant(qr_all[:, 2 * c : 2 * (c + 1), :], rf, rs_bc)

        # m0/m1 weights quantize from resident lexc inside the rhs
        # collective window (only ls is needed; no DMA involved).
        ql0 = ql_tile(0)
        ql1 = ql_tile(1)
        for c in range(2):
            ql_chunk_resident(ql0, 0, c)
        for c in range(2):
            ql_chunk_resident(ql1, 1, c)
        for c in range(2, 4):
            ql_chunk_resident(ql0, 0, c)
        for c in range(2, 4):
            ql_chunk_resident(ql1, 1, c)

        # PE warm-up bridge: matmuls on already-quantized (garbage-free)
        # weight data into a scratch psum bank, spanning the window between
        # the weight quantize and the first real matmul so the PE clock is
        # fully ramped when the burst starts.  Results are never read.
        wps = psum.tile([P, 512], f32, tag="ps", name="warm")
        for w in range(60):
            nc.tensor.matmul(
                wps[:, 0:256],
                lhsT=ql1[:, 31, 0:P],
                rhs=ql1[:, w % KT, :],
                start=True,
                stop=True,
            )

        # rs, then the local-half qr re-read (first 3 chunks prefetched)
        cc_finish(gr, rs_bc, 1)
        for h in range(2):
            quant(
                qr_all[:, h : h + 1, :], qr_pre[0][:, h : h + 1, :], rs_bc
            )
        for c in range(1, 3):
            quant(qr_all[:, 2 * c : 2 * (c + 1), :], qr_pre[c], rs_bc)
        for c in range(3, 8):
            qr_chunk(c)

        # d = 1/(ls*rs), Newton-polished (first consumed by macro-0 dequant)
        p_t = singles.tile([P, 1], f32)
        d_t = singles.tile([P, 1], f32)
        u_t = singles.tile([P, 1], f32)
        nc.vector.tensor_mul(p_t, lsrs[:, 0:1], lsrs[:, 1:2])
        nc.vector.reciprocal(d_t, p_t)
        nc.vector.tensor_mul(u_t, p_t, d_t)
        nc.vector.tensor_scalar(u_t, u_t, -1.0, 2.0, op0=OP.mult, op1=OP.add)
        nc.vector.tensor_mul(d_t, d_t, u_t)
        d_bc = d_t[:, 0:1]

        # --- matmul helpers ---
        def mk_psum(m):
            return [
                psum.tile([P, 512], f32, tag="ps", name=f"ps{m}_{q}")
                for q in range(4)
            ]

        def mm_k(ql, pst, k, st, sp):
            for ms in range(2):
                w = ql[:, k, ms * P : (ms + 1) * P]
                nc.tensor.matmul(
                    pst[2 * ms], lhsT=w, rhs=qr_all[:, k, 0:512],
                    start=st, stop=sp,
                )
                nc.tensor.matmul(
                    pst[2 * ms + 1], lhsT=w, rhs=qr_all[:, k, 512:1024],
                    start=st, stop=sp,
                )

        def dequant_out(pst, m, ms_range=(0, 1), eng=None):
            # dequant on ACT: it is idle in the steady state (one mul pass
            # per macro) while DVE carries the round stream, and ACT reads
            # PSUM directly, so the scale-by-d runs right after the bank's
            # stop-matmul instead of queueing behind DVE work.
            eng = eng or nc.gpsimd
            for ms in ms_range:
                for h in range(2):
                    osb = outp.tile([P, 512], f32)
                    nc.scalar.mul(out=osb, in_=pst[2 * ms + h], mul=d_bc)
                    eng.dma_start(
                        out=out_v[m * 2 + ms, :, 512 * h : 512 * (h + 1)],
                        in_=osb,
                    )

        # --- m0/m1: local k-half first, then remote with the qr stream ---
        pst0 = mk_psum(0)
        for k in range(KH):
            mm_k(ql0, pst0, k, k == 0, False)
        pst1 = mk_psum(1)
        for k in range(KH):
            mm_k(ql1, pst1, k, k == 0, False)

        # remote rhs half
        for c in range(8, 16):
            qr_chunk(c)

        for k in range(KH, KT):
            mm_k(ql0, pst0, k, False, k == KT - 1)
        for k in range(KH, KT):
            mm_k(ql1, pst1, k, False, k == KT - 1)

        # --- macros 2..6: stream lhsT, quantize, matmul.  Each macro's
        # weight quantize is emitted BEFORE the previous macros' dequant so
        # the DVE serves the PE's critical input first; psum stays within 8
        # banks because the deferred dequant still precedes the next
        # macro's matmuls.
        pending = [(pst0, 0), (pst1, 1)]
        for mt in range(2, NMACRO - 1):
            ql = ql_tile(mt)
            for j in range(4):
                ql_chunk_stream(ql, mt, j)
            for pq, pm in pending:
                dequant_out(pq, pm, eng=nc.sync if pm >= NMACRO - 2 else None)
            pending = []
            pst = mk_psum(mt)
            for k in range(KT):
                mm_k(ql, pst, k, k == 0, k == KT - 1)
            pending.append((pst, mt))

        # --- macro 7: the two output halves run sequentially so the first
        # half's dequant + store hides under the second half's matmuls ---
        mt = NMACRO - 1
        ql = ql_tile(mt)
        for j in range(4):
            ql_chunk_stream(ql, mt, j)
        for pq, pm in pending:
            dequant_out(pq, pm, eng=nc.sync)
        pending = []
        pst = mk_psum(mt)
        for k in range(KT):
            ms = 0
            w = ql[:, k, ms * P : (ms + 1) * P]
            nc.tensor.matmul(pst[0], lhsT=w, rhs=qr_all[:, k, 0:512],
                             start=k == 0, stop=k == KT - 1)
            nc.tensor.matmul(pst[1], lhsT=w, rhs=qr_all[:, k, 512:1024],
                             start=k == 0, stop=k == KT - 1)
        dequant_out(pst, mt, ms_range=(0,), eng=nc.sync)
        for k in range(KT):
            ms = 1
            w = ql[:, k, ms * P : (ms + 1) * P]
            nc.tensor.matmul(pst[2], lhsT=w, rhs=qr_all[:, k, 0:512],
                             start=k == 0, stop=k == KT - 1)
            nc.tensor.matmul(pst[3], lhsT=w, rhs=qr_all[:, k, 512:1024],
                             start=k == 0, stop=k == KT - 1)
        dequant_out(pst, mt, ms_range=(1,), eng=nc.sync)

    nc.compile()
    return nc


def _get_program():
    global _cached
    if _cached is None:
        _cached = _build_program()
    return _cached


def _mperm(ci):
    sl = ci * 512
    return np.concatenate(
        [
            np.arange(sl, sl + 512),
            np.arange(0, sl),
            np.arange(sl + 512, MB),
        ]
    )


def _shard_inputs(lhs, rhs):
    lhs = np.ascontiguousarray(np.asarray(lhs, dtype=np.float32))
    rhs = np.ascontiguousarray(np.asarray(rhs, dtype=np.float32))
    assert lhs.shape == (M, K) and rhs.shape == (K, N)
    lhsT = np.ascontiguousarray(lhs.T)  # [K, M]
    in_maps = []
    for i in range(N_CORES):
        ri, ci = divmod(i, CG)
        lT = lhsT[:, ri * MB : (ri + 1) * MB]
        rsh = rhs[:, ci * NB : (ci + 1) * NB]
        # roll k so the core's stats k-half (rows [ri*MB,(ri+1)*MB)) is first
        if ri:
            lT = np.concatenate([lT[MB:], lT[:MB]], axis=0)
            rsh = np.concatenate([rsh[MB:], rsh[:MB]], axis=0)
        # permute lhsT cols so the core's stats slice (ci-th 512) is first
        lT = np.ascontiguousarray(lT[:, _mperm(ci)])
        rsh = np.ascontiguousarray(rsh)
        in_maps.append({"lhsT": lT, "rhs": rsh})
    return in_maps


def _gather(results):
    out = np.empty((M, N), dtype=np.float32)
    for i in range(N_CORES):
        ri, ci = divmod(i, CG)
        rows = ri * MB + _mperm(ci)
        out[rows, ci * NB : (ci + 1) * NB] = results[i]["out"]
    return out


def run(lhs, rhs, trace=False):
    """Run the kernel; returns (out, BassKernelResults)."""
    from concourse import bass_utils

    nc = _get_program()
    in_maps = _shard_inputs(lhs, rhs)
    res = bass_utils.run_bass_kernel_spmd(
        nc, in_maps, core_ids=list(range(N_CORES)), trace=trace
    )
    return _gather(res.results), res


def kernel(lhs, rhs):
    out, _ = run(lhs, rhs, trace=False)
    return out
